# revision 26
# baseline (speedup 1.0000x reference)
"""AdaFace logits kernel for 8 TRN2 NeuronCores.

Math (reference): kernel columns are L2-normalized, cosine = clip(emb @ kn,
-1+eps, 1-eps), then an angular margin is applied *only at the label column
of each row* (for every other column cos(clip(arccos(clip(c)))) == c since
the theta-clip cannot bind once |c| <= 1-eps), finally scaled by S=64.

Strategy: shard kernel/output column-wise across 8 cores (tensor/vocab
parallel, no collectives needed). Compute in bf16 (rel err ~3e-3, well
under the 2e-2 gate). Per core:
  - out_T[cols, batch] = kernel_shard^T @ emb^T via bf16 matmuls
    (separate FWL weight loads overlap with the matmul stream)
  - column norms: square kernel tiles on VectorE (bf16 2x), reduce over
    the contraction axis with tiny [128cols,1] matmuls against a ones
    vector so the norm lands on the partition axis
  - epilogue: ScalarE Copy PSUM->SBUF fused with the per-partition
    64/norm scale, one dual-op VectorE clip to +/-(1-eps)*64 over the
    whole group, single 0.5 MB output DMA per group
  - label margin: the host shards the labels (each label falls in
    exactly one column shard); each core receives its compacted in-shard
    fix list (flat output index + that row's feature norm, padded to 128
    with OOB indices).  The margin scalars (mean/std over all 512 norms)
    are computed on device; the tail does ONE masked indirect-DMA gather
    of the label logits from the output, the closed-form cos(theta+g)
    fix (no arccos needed: cos(t+g) = c*cos g - sqrt(1-c^2)*sin g, with
    the theta-clip handled by per-slot threshold compares), and ONE
    masked indirect-DMA scatter back (OOB-padded slots are skipped).

Host only shards/pads/casts inputs, transposes emb once, and
transposes/concats/upcasts the output shards back to (512, 100000) f32.
"""

import math
import os
import sys

for _p in ("/opt/trn_rl_repo",):
    if _p not in sys.path and os.path.isdir(_p):
        sys.path.append(_p)

import numpy as np

import concourse.mybir as mybir
import concourse.tile as tile
from concourse import bacc
from concourse.bass import IndirectOffsetOnAxis
from concourse.bass_utils import run_bass_kernel_spmd

F32 = mybir.dt.float32
BF16 = mybir.dt.bfloat16
I32 = mybir.dt.int32
AF = mybir.ActivationFunctionType
OP = mybir.AluOpType

N = 512          # batch rows
EMB = 512        # embedding dim
C = 100000       # num classes
NCORES = 8
CSH = C // NCORES          # 12500 real cols per core
CPAD = 12544               # padded cols per core: 24 groups of 512 + 256
NFULL = 24                 # full 512-col groups
EPS = 0.001
MARGIN = 0.4
H = 0.333
S = 64.0
CLIP = (1.0 - EPS) * S     # 63.936
COS_EPS = math.cos(EPS)

_CACHE = {}
LAST_RESULTS = None


def _build():
    nc = bacc.Bacc("TRN2", target_bir_lowering=False, debug=False,
                   num_devices=NCORES)
    kern = nc.dram_tensor("kern", [EMB, CPAD], BF16, kind="ExternalInput")
    embT = nc.dram_tensor("embT", [EMB, N], BF16, kind="ExternalInput")
    normv = nc.dram_tensor("normv", [N], F32, kind="ExternalInput")
    fix_flat = nc.dram_tensor("fix_flat", [128, 1], I32, kind="ExternalInput")
    fix_norm = nc.dram_tensor("fix_norm", [128, 1], F32, kind="ExternalInput")
    out = nc.dram_tensor("out", [CPAD, N], BF16, kind="ExternalOutput")

    kern_r = kern.ap().rearrange("(t p) c -> p t c", p=128)
    embT_r = embT.ap().rearrange("(t p) n -> p t n", p=128)
    norm_row_r = normv.ap().rearrange("(a b) -> a b", a=1)
    out_flat = out.ap().rearrange("a (b c) -> (a b) c", c=1)
    out_g = out.ap()[0:NFULL * 512, :].rearrange(
        "(g c p) n -> g p c n", p=128, c=4)
    out_gh = out.ap()[NFULL * 512:CPAD, :].rearrange(
        "(g c p) n -> g p c n", p=128, c=2)

    with tile.TileContext(nc) as tc:
        with (
            tc.tile_pool(name="persist", bufs=1) as pp,
            tc.tile_pool(name="kt", bufs=6) as kt_pool,
            tc.tile_pool(name="sq", bufs=4) as sq_pool,
            tc.tile_pool(name="ob", bufs=5) as ob_pool,
            tc.tile_pool(name="rs", bufs=2) as rs_pool,
            tc.tile_pool(name="pmain", bufs=7, space="PSUM") as pmain,
            tc.tile_pool(name="pns", bufs=1, space="PSUM") as pns,
        ):
            # ---- persistent inputs (emitted first so matmuls start early)
            embT_sb = pp.tile([128, 4, N], BF16)
            nc.sync.dma_start(embT_sb[:], embT_r[:])
            ones_bf = pp.tile([128, 1], BF16)
            nc.vector.memset(ones_bf[:], 1.0)
            ones_row = pp.tile([1, 128], F32)
            nc.vector.memset(ones_row[:], 1.0)

            def group(g, nsub, ob_view, ob_tag, kt_tag, sq_tag):
                ncols = nsub * 128
                ktile = kt_pool.tile([128, 4, ncols], BF16, tag=kt_tag)
                nc.sync.dma_start(
                    ktile[:], kern_r[:, :, g * 512:g * 512 + ncols])
                sq = sq_pool.tile([128, 4, ncols], BF16, tag=sq_tag)
                nc.vector.tensor_tensor(out=sq[:], in0=ktile[:],
                                        in1=ktile[:], op=OP.mult)
                ns_ps = pns.tile([128, 4], F32, tag="ns")
                mains = []
                for c in range(nsub):
                    mm = pmain.tile([128, 512], F32, tag="mm")
                    for t in range(4):
                        nc.tensor.matmul(
                            mm[:],
                            lhsT=ktile[:, t, c * 128:(c + 1) * 128],
                            rhs=embT_sb[:, t, :],
                            start=(t == 0), stop=(t == 3))
                    for t in range(4):
                        nc.tensor.matmul(
                            ns_ps[:, c:c + 1],
                            lhsT=sq[:, t, c * 128:(c + 1) * 128],
                            rhs=ones_bf[:],
                            start=(t == 0), stop=(t == 3))
                    mains.append(mm)
                rs = rs_pool.tile([128, 4], F32, tag="rs")
                nc.scalar.activation(rs[:], ns_ps[:], AF.Sqrt,
                                     scale=1.0 / (S * S))   # norm/64
                nc.vector.reciprocal(rs[:], rs[:])           # 64/norm
                ob = ob_pool.tile([128, nsub, 512], BF16, tag=ob_tag)
                for c in range(nsub):
                    nc.scalar.activation(ob[:, c, :], mains[c][:], AF.Copy,
                                         scale=rs[:, c:c + 1])
                nc.vector.tensor_scalar(out=ob[:], in0=ob[:],
                                        scalar1=CLIP, scalar2=-CLIP,
                                        op0=OP.min, op1=OP.max)
                nc.sync.dma_start(ob_view, ob[:])

            # group 0 first so the PE pipeline starts immediately
            group(0, 4, out_g[0], "ob", "kt", "sq")

            # ---- margin scalars from norms (mean / unbiased std) -------
            nr = pp.tile([1, N], F32)
            nc.sync.dma_start(nr[:], norm_row_r[:])
            nrc = pp.tile([1, N], F32)
            nc.vector.tensor_scalar(out=nrc[:], in0=nr[:], scalar1=0.001,
                                    scalar2=100.0, op0=OP.max, op1=OP.min)
            s1 = pp.tile([1, 1], F32)
            nc.vector.tensor_reduce(s1[:], nrc[:], axis=mybir.AxisListType.X,
                                    op=OP.add)
            sqr = pp.tile([1, N], F32)
            s2 = pp.tile([1, 1], F32)
            nc.scalar.activation(sqr[:], nrc[:], AF.Square, accum_out=s2[:])
            mean = pp.tile([1, 1], F32)
            nc.vector.tensor_scalar_mul(mean[:], s1[:], 1.0 / N)
            var = pp.tile([1, 1], F32)
            nc.vector.tensor_tensor(out=var[:], in0=mean[:], in1=mean[:],
                                    op=OP.mult)
            nc.vector.tensor_scalar_mul(var[:], var[:], -float(N))
            nc.vector.tensor_tensor(out=var[:], in0=var[:], in1=s2[:],
                                    op=OP.add)
            nc.vector.tensor_scalar_mul(var[:], var[:], 1.0 / (N - 1))
            std = pp.tile([1, 1], F32)
            nc.scalar.activation(std[:], var[:], AF.Sqrt)
            denom = pp.tile([1, 1], F32)
            nc.vector.tensor_scalar_add(denom[:], std[:], EPS)
            inv = pp.tile([1, 1], F32)
            nc.vector.reciprocal(inv[:], denom[:])
            a_sc = pp.tile([1, 1], F32)
            nc.vector.tensor_scalar_mul(a_sc[:], inv[:], H)
            mb = pp.tile([1, 2], F32)
            nc.vector.tensor_copy(mb[:, 0:1], mean[:])
            nc.vector.tensor_copy(mb[:, 1:2], a_sc[:])
            bc_ps = pns.tile([128, 2], F32, tag="ns")
            nc.tensor.matmul(bc_ps[:], lhsT=ones_row[:], rhs=mb[:],
                             start=True, stop=True)
            bc_sb = pp.tile([128, 2], F32)
            nc.scalar.activation(bc_sb[:], bc_ps[:], AF.Copy)

            # ---- per-slot margin tiles (host-sharded label fix list) ----
            npc = pp.tile([128, 1], F32)
            nc.sync.dma_start(npc[:], fix_norm.ap())
            ms = pp.tile([128, 1], F32)
            nc.vector.tensor_scalar(out=ms[:], in0=npc[:], scalar1=0.001,
                                    scalar2=100.0, op0=OP.max, op1=OP.min)
            nc.vector.tensor_scalar(out=ms[:], in0=ms[:],
                                    scalar1=bc_sb[:, 0:1],
                                    scalar2=bc_sb[:, 1:2],
                                    op0=OP.subtract, op1=OP.mult)
            nc.vector.tensor_scalar(out=ms[:], in0=ms[:], scalar1=-1.0,
                                    scalar2=1.0, op0=OP.max, op1=OP.min)
            g_t = pp.tile([128, 1], F32)
            nc.vector.tensor_scalar_mul(g_t[:], ms[:], -MARGIN)
            b_pi2 = pp.tile([128, 1], F32)
            nc.vector.memset(b_pi2[:], math.pi / 2)
            b_pie = pp.tile([128, 1], F32)
            nc.vector.memset(b_pie[:], math.pi / 2 + EPS)
            sing = pp.tile([128, 1], F32)
            nc.scalar.activation(sing[:], g_t[:], AF.Sin, bias=0.0)
            cosg = pp.tile([128, 1], F32)
            nc.scalar.activation(cosg[:], g_t[:], AF.Sin, bias=b_pi2[:])
            tlo = pp.tile([128, 1], F32)   # cos(EPS - g)
            nc.scalar.activation(tlo[:], g_t[:], AF.Sin,
                                 bias=b_pie[:], scale=-1.0)
            thi = pp.tile([128, 1], F32)   # -cos(EPS + g)
            nc.scalar.activation(thi[:], g_t[:], AF.Sin, bias=b_pie[:])
            nc.vector.tensor_scalar_mul(thi[:], thi[:], -1.0)
            gadd64 = pp.tile([128, 1], F32)  # 64*(M - g)
            nc.vector.tensor_scalar(out=gadd64[:], in0=g_t[:], scalar1=-S,
                                    scalar2=S * MARGIN, op0=OP.mult,
                                    op1=OP.add)
            flat_m = pp.tile([128, 1], I32)
            nc.sync.dma_start(flat_m[:], fix_flat.ap())

            # ---- remaining groups --------------------------------------
            for g in range(1, NFULL):
                group(g, 4, out_g[g], "ob", "kt", "sq")
            group(NFULL, 2, out_gh[0], "obh", "kth", "sqh")

            # ---- label-column margin fix (gather -> fix -> scatter) ----
            gth = pp.tile([128, 1], BF16)
            nc.vector.memset(gth[:], 0.5)
            nc.gpsimd.indirect_dma_start(
                out=gth[:], out_offset=None,
                in_=out_flat[:],
                in_offset=IndirectOffsetOnAxis(ap=flat_m[:], axis=0),
                bounds_check=CPAD * N - 1,
                oob_is_err=False)
            cc = pp.tile([128, 1], F32)
            nc.vector.tensor_scalar_mul(cc[:], gth[:], 1.0 / S)
            v2 = pp.tile([128, 1], F32)
            nc.scalar.activation(v2[:], cc[:], AF.Square)
            sn = pp.tile([128, 1], F32)
            nc.scalar.activation(sn[:], v2[:], AF.Sqrt, scale=-1.0, bias=1.0)
            t1 = pp.tile([128, 1], F32)
            nc.vector.tensor_tensor(out=t1[:], in0=cc[:], in1=cosg[:],
                                    op=OP.mult)
            t2 = pp.tile([128, 1], F32)
            nc.vector.tensor_tensor(out=t2[:], in0=sn[:], in1=sing[:],
                                    op=OP.mult)
            cm = pp.tile([128, 1], F32)
            nc.vector.tensor_tensor(out=cm[:], in0=t1[:], in1=t2[:],
                                    op=OP.subtract)
            clo = pp.tile([128, 1], I32)
            nc.vector.tensor_tensor(out=clo[:], in0=cc[:], in1=tlo[:],
                                    op=OP.is_gt)
            chi = pp.tile([128, 1], I32)
            nc.vector.tensor_tensor(out=chi[:], in0=cc[:], in1=thi[:],
                                    op=OP.is_lt)
            ce_p = pp.tile([128, 1], F32)
            nc.vector.memset(ce_p[:], COS_EPS)
            ce_n = pp.tile([128, 1], F32)
            nc.vector.memset(ce_n[:], -COS_EPS)
            cm2 = pp.tile([128, 1], F32)
            nc.vector.select(cm2[:], clo[:], ce_p[:], cm[:])
            cm3 = pp.tile([128, 1], F32)
            nc.vector.select(cm3[:], chi[:], ce_n[:], cm2[:])
            w_in = pp.tile([128, 1], F32)
            nc.vector.tensor_scalar_mul(w_in[:], cm3[:], S)
            nc.vector.tensor_tensor(out=w_in[:], in0=w_in[:], in1=gadd64[:],
                                    op=OP.subtract)
            w_bf = pp.tile([128, 1], BF16)
            nc.vector.tensor_copy(w_bf[:], w_in[:])
            scat_sem = nc.alloc_semaphore("scat_sem")
            with tc.tile_critical(no_gpsimd_drain=True):
                nc.gpsimd.indirect_dma_start(
                    out=out_flat[:],
                    out_offset=IndirectOffsetOnAxis(ap=flat_m[:], axis=0),
                    in_=w_bf[:], in_offset=None,
                    bounds_check=CPAD * N - 1,
                    oob_is_err=False).then_inc(scat_sem, 16)

    nc.compile()
    return nc


def _get_nc():
    if "nc" not in _CACHE:
        _CACHE["nc"] = _build()
    return _CACHE["nc"]


def kernel(embeddings, kernel, norms, label):
    global LAST_RESULTS
    import ml_dtypes

    emb = np.asarray(embeddings, dtype=np.float32)
    kern_f = np.asarray(kernel, dtype=np.float32)
    norms_f = np.asarray(norms, dtype=np.float32)
    lab = np.asarray(label).astype(np.int64)

    nc = _get_nc()

    embT = np.ascontiguousarray(emb.T).astype(ml_dtypes.bfloat16)
    kern_bf = kern_f.astype(ml_dtypes.bfloat16)
    in_maps = []
    for i in range(NCORES):
        sh = np.ones((EMB, CPAD), dtype=ml_dtypes.bfloat16)
        sh[:, :CSH] = kern_bf[:, i * CSH:(i + 1) * CSH]
        c0 = i * CSH
        rows = np.where((lab >= c0) & (lab < c0 + CSH))[0]
        assert len(rows) <= 128, "fix list overflows one partition tile"
        ff = np.full((128, 1), 2 ** 30, dtype=np.int32)
        fn = np.full((128, 1), 10.0, dtype=np.float32)
        ff[:len(rows), 0] = ((lab[rows] - c0) * N + rows).astype(np.int32)
        fn[:len(rows), 0] = norms_f[rows]
        in_maps.append({
            "kern": sh,
            "embT": embT,
            "normv": norms_f,
            "fix_flat": ff,
            "fix_norm": fn,
        })

    trace = bool(os.environ.get("KBENCH_TRACE"))
    try:
        res = run_bass_kernel_spmd(nc, in_maps, core_ids=list(range(NCORES)),
                                   trace=trace)
    except ModuleNotFoundError:
        res = run_bass_kernel_spmd(nc, in_maps, core_ids=list(range(NCORES)))
    LAST_RESULTS = res

    scaled = np.empty((N, C), dtype=np.float32)
    for i in range(NCORES):
        scaled[:, i * CSH:(i + 1) * CSH] = \
            res.results[i]["out"][:CSH, :].astype(np.float32).T
    return scaled, emb


# revision 27
# speedup vs baseline: 1.0182x; 1.0182x over previous
"""AdaFace logits kernel for 8 TRN2 NeuronCores.

Math (reference): kernel columns are L2-normalized, cosine = clip(emb @ kn,
-1+eps, 1-eps), then an angular margin is applied *only at the label column
of each row* (for every other column cos(clip(arccos(clip(c)))) == c since
the theta-clip cannot bind once |c| <= 1-eps), finally scaled by S=64.

Strategy: shard kernel/output column-wise across 8 cores (tensor/vocab
parallel, no collectives needed). Compute in bf16 (rel err ~3e-3, well
under the 2e-2 gate). Per core:
  - out_T[cols, batch] = kernel_shard^T @ emb^T via bf16 matmuls
    (separate FWL weight loads overlap with the matmul stream)
  - column norms: square kernel tiles on VectorE (bf16 2x), reduce over
    the contraction axis with tiny [128cols,1] matmuls against a ones
    vector so the norm lands on the partition axis
  - epilogue: ScalarE Copy PSUM->SBUF fused with the per-partition
    64/norm scale, one dual-op VectorE clip to +/-(1-eps)*64 over the
    whole group, single 0.5 MB output DMA per group
  - label margin: the host shards the labels (each label falls in
    exactly one column shard); each core receives its compacted in-shard
    fix list (flat output index + that row's feature norm, padded to 128
    with OOB indices).  The margin scalars (mean/std over all 512 norms)
    are computed on device; the tail does ONE masked indirect-DMA gather
    of the label logits from the output, the closed-form cos(theta+g)
    fix (no arccos needed: cos(t+g) = c*cos g - sqrt(1-c^2)*sin g, with
    the theta-clip handled by per-slot threshold compares), and ONE
    masked indirect-DMA scatter back (OOB-padded slots are skipped).

Host only shards/pads/casts inputs, transposes emb once, and
transposes/concats/upcasts the output shards back to (512, 100000) f32.
"""

import math
import os
import sys

for _p in ("/opt/trn_rl_repo",):
    if _p not in sys.path and os.path.isdir(_p):
        sys.path.append(_p)

import numpy as np

import concourse.mybir as mybir
import concourse.tile as tile
from concourse import bacc
from concourse.bass import IndirectOffsetOnAxis
from concourse.bass_utils import run_bass_kernel_spmd

F32 = mybir.dt.float32
BF16 = mybir.dt.bfloat16
I32 = mybir.dt.int32
AF = mybir.ActivationFunctionType
OP = mybir.AluOpType

N = 512          # batch rows
EMB = 512        # embedding dim
C = 100000       # num classes
NCORES = 8
CSH = C // NCORES          # 12500 real cols per core
CPAD = 12544               # padded cols per core: 24 groups of 512 + 256
NFULL = 24                 # full 512-col groups
EPS = 0.001
MARGIN = 0.4
H = 0.333
S = 64.0
CLIP = (1.0 - EPS) * S     # 63.936
COS_EPS = math.cos(EPS)

_CACHE = {}
LAST_RESULTS = None


def _build():
    nc = bacc.Bacc("TRN2", target_bir_lowering=False, debug=False,
                   num_devices=NCORES)
    kern = nc.dram_tensor("kern", [EMB, CPAD], BF16, kind="ExternalInput")
    embT = nc.dram_tensor("embT", [EMB, N], BF16, kind="ExternalInput")
    normv = nc.dram_tensor("normv", [N], F32, kind="ExternalInput")
    fix_flat = nc.dram_tensor("fix_flat", [128, 1], I32, kind="ExternalInput")
    fix_norm = nc.dram_tensor("fix_norm", [128, 1], F32, kind="ExternalInput")
    out = nc.dram_tensor("out", [CPAD, N], BF16, kind="ExternalOutput")

    kern_r = kern.ap().rearrange("(t p) c -> p t c", p=128)
    embT_r = embT.ap().rearrange("(t p) n -> p t n", p=128)
    norm_row_r = normv.ap().rearrange("(a b) -> a b", a=1)
    out_flat = out.ap().rearrange("a (b c) -> (a b) c", c=1)
    out_g = out.ap()[0:NFULL * 512, :].rearrange(
        "(g c p) n -> g p c n", p=128, c=4)
    out_gh = out.ap()[NFULL * 512:CPAD, :].rearrange(
        "(g c p) n -> g p c n", p=128, c=2)

    with tile.TileContext(nc) as tc:
        with (
            tc.tile_pool(name="persist", bufs=1) as pp,
            tc.tile_pool(name="kt", bufs=6) as kt_pool,
            tc.tile_pool(name="sq", bufs=4) as sq_pool,
            tc.tile_pool(name="ob", bufs=5) as ob_pool,
            tc.tile_pool(name="rs", bufs=2) as rs_pool,
            tc.tile_pool(name="pmain", bufs=6, space="PSUM") as pmain,
            tc.tile_pool(name="pns", bufs=2, space="PSUM") as pns,
        ):
            # ---- persistent inputs (emitted first so matmuls start early)
            embT_sb = pp.tile([128, 4, N], BF16)
            nc.sync.dma_start(embT_sb[:], embT_r[:])
            ones_bf = pp.tile([128, 1], BF16)
            nc.vector.memset(ones_bf[:], 1.0)
            ones_row = pp.tile([1, 128], F32)
            nc.vector.memset(ones_row[:], 1.0)

            def group(g, nsub, ob_view, ob_tag, kt_tag, sq_tag):
                ncols = nsub * 128
                ktile = kt_pool.tile([128, 4, ncols], BF16, tag=kt_tag)
                nc.sync.dma_start(
                    ktile[:], kern_r[:, :, g * 512:g * 512 + ncols])
                sq = sq_pool.tile([128, 4, ncols], BF16, tag=sq_tag)
                nc.vector.tensor_tensor(out=sq[:], in0=ktile[:],
                                        in1=ktile[:], op=OP.mult)
                ns_ps = pns.tile([128, 4], F32, tag="ns")
                mains = []
                for c in range(nsub):
                    mm = pmain.tile([128, 512], F32, tag="mm")
                    for t in range(4):
                        nc.tensor.matmul(
                            mm[:],
                            lhsT=ktile[:, t, c * 128:(c + 1) * 128],
                            rhs=embT_sb[:, t, :],
                            start=(t == 0), stop=(t == 3))
                    for t in range(4):
                        nc.tensor.matmul(
                            ns_ps[:, c:c + 1],
                            lhsT=sq[:, t, c * 128:(c + 1) * 128],
                            rhs=ones_bf[:],
                            start=(t == 0), stop=(t == 3))
                    mains.append(mm)
                rs = rs_pool.tile([128, 4], F32, tag="rs")
                nc.scalar.activation(rs[:], ns_ps[:], AF.Sqrt,
                                     scale=1.0 / (S * S))   # norm/64
                nc.vector.reciprocal(rs[:], rs[:])           # 64/norm
                ob = ob_pool.tile([128, nsub, 512], BF16, tag=ob_tag)
                for c in range(nsub):
                    nc.scalar.activation(ob[:, c, :], mains[c][:], AF.Copy,
                                         scale=rs[:, c:c + 1])
                nc.vector.tensor_scalar(out=ob[:], in0=ob[:],
                                        scalar1=CLIP, scalar2=-CLIP,
                                        op0=OP.min, op1=OP.max)
                nc.sync.dma_start(ob_view, ob[:])

            # group 0 first so the PE pipeline starts immediately
            group(0, 4, out_g[0], "ob", "kt", "sq")

            # ---- margin scalars from norms (mean / unbiased std) -------
            nr = pp.tile([1, N], F32)
            nc.sync.dma_start(nr[:], norm_row_r[:])
            nrc = pp.tile([1, N], F32)
            nc.vector.tensor_scalar(out=nrc[:], in0=nr[:], scalar1=0.001,
                                    scalar2=100.0, op0=OP.max, op1=OP.min)
            s1 = pp.tile([1, 1], F32)
            nc.vector.tensor_reduce(s1[:], nrc[:], axis=mybir.AxisListType.X,
                                    op=OP.add)
            sqr = pp.tile([1, N], F32)
            s2 = pp.tile([1, 1], F32)
            nc.scalar.activation(sqr[:], nrc[:], AF.Square, accum_out=s2[:])
            mean = pp.tile([1, 1], F32)
            nc.vector.tensor_scalar_mul(mean[:], s1[:], 1.0 / N)
            var = pp.tile([1, 1], F32)
            nc.vector.tensor_tensor(out=var[:], in0=mean[:], in1=mean[:],
                                    op=OP.mult)
            nc.vector.tensor_scalar_mul(var[:], var[:], -float(N))
            nc.vector.tensor_tensor(out=var[:], in0=var[:], in1=s2[:],
                                    op=OP.add)
            nc.vector.tensor_scalar_mul(var[:], var[:], 1.0 / (N - 1))
            std = pp.tile([1, 1], F32)
            nc.scalar.activation(std[:], var[:], AF.Sqrt)
            denom = pp.tile([1, 1], F32)
            nc.vector.tensor_scalar_add(denom[:], std[:], EPS)
            inv = pp.tile([1, 1], F32)
            nc.vector.reciprocal(inv[:], denom[:])
            a_sc = pp.tile([1, 1], F32)
            nc.vector.tensor_scalar_mul(a_sc[:], inv[:], H)
            mb = pp.tile([1, 2], F32)
            nc.vector.tensor_copy(mb[:, 0:1], mean[:])
            nc.vector.tensor_copy(mb[:, 1:2], a_sc[:])
            bc_ps = pns.tile([128, 2], F32, tag="ns")
            nc.tensor.matmul(bc_ps[:], lhsT=ones_row[:], rhs=mb[:],
                             start=True, stop=True)
            bc_sb = pp.tile([128, 2], F32)
            nc.scalar.activation(bc_sb[:], bc_ps[:], AF.Copy)

            # ---- per-slot margin tiles (host-sharded label fix list) ----
            npc = pp.tile([128, 1], F32)
            nc.sync.dma_start(npc[:], fix_norm.ap())
            ms = pp.tile([128, 1], F32)
            nc.vector.tensor_scalar(out=ms[:], in0=npc[:], scalar1=0.001,
                                    scalar2=100.0, op0=OP.max, op1=OP.min)
            nc.vector.tensor_scalar(out=ms[:], in0=ms[:],
                                    scalar1=bc_sb[:, 0:1],
                                    scalar2=bc_sb[:, 1:2],
                                    op0=OP.subtract, op1=OP.mult)
            nc.vector.tensor_scalar(out=ms[:], in0=ms[:], scalar1=-1.0,
                                    scalar2=1.0, op0=OP.max, op1=OP.min)
            g_t = pp.tile([128, 1], F32)
            nc.vector.tensor_scalar_mul(g_t[:], ms[:], -MARGIN)
            b_pi2 = pp.tile([128, 1], F32)
            nc.vector.memset(b_pi2[:], math.pi / 2)
            b_pie = pp.tile([128, 1], F32)
            nc.vector.memset(b_pie[:], math.pi / 2 + EPS)
            sing = pp.tile([128, 1], F32)
            nc.scalar.activation(sing[:], g_t[:], AF.Sin, bias=0.0)
            cosg = pp.tile([128, 1], F32)
            nc.scalar.activation(cosg[:], g_t[:], AF.Sin, bias=b_pi2[:])
            tlo = pp.tile([128, 1], F32)   # cos(EPS - g)
            nc.scalar.activation(tlo[:], g_t[:], AF.Sin,
                                 bias=b_pie[:], scale=-1.0)
            thi = pp.tile([128, 1], F32)   # -cos(EPS + g)
            nc.scalar.activation(thi[:], g_t[:], AF.Sin, bias=b_pie[:])
            nc.vector.tensor_scalar_mul(thi[:], thi[:], -1.0)
            gadd64 = pp.tile([128, 1], F32)  # 64*(M - g)
            nc.vector.tensor_scalar(out=gadd64[:], in0=g_t[:], scalar1=-S,
                                    scalar2=S * MARGIN, op0=OP.mult,
                                    op1=OP.add)
            flat_m = pp.tile([128, 1], I32)
            nc.sync.dma_start(flat_m[:], fix_flat.ap())

            # ---- remaining groups --------------------------------------
            for g in range(1, NFULL):
                group(g, 4, out_g[g], "ob", "kt", "sq")
            group(NFULL, 2, out_gh[0], "obh", "kth", "sqh")

            # ---- label-column margin fix (gather -> fix -> scatter) ----
            gth = pp.tile([128, 1], BF16)
            nc.vector.memset(gth[:], 0.5)
            nc.gpsimd.indirect_dma_start(
                out=gth[:], out_offset=None,
                in_=out_flat[:],
                in_offset=IndirectOffsetOnAxis(ap=flat_m[:], axis=0),
                bounds_check=CPAD * N - 1,
                oob_is_err=False)
            cc = pp.tile([128, 1], F32)
            nc.vector.tensor_scalar_mul(cc[:], gth[:], 1.0 / S)
            v2 = pp.tile([128, 1], F32)
            nc.scalar.activation(v2[:], cc[:], AF.Square)
            sn = pp.tile([128, 1], F32)
            nc.scalar.activation(sn[:], v2[:], AF.Sqrt, scale=-1.0, bias=1.0)
            t1 = pp.tile([128, 1], F32)
            nc.vector.tensor_tensor(out=t1[:], in0=cc[:], in1=cosg[:],
                                    op=OP.mult)
            t2 = pp.tile([128, 1], F32)
            nc.vector.tensor_tensor(out=t2[:], in0=sn[:], in1=sing[:],
                                    op=OP.mult)
            cm = pp.tile([128, 1], F32)
            nc.vector.tensor_tensor(out=cm[:], in0=t1[:], in1=t2[:],
                                    op=OP.subtract)
            clo = pp.tile([128, 1], I32)
            nc.vector.tensor_tensor(out=clo[:], in0=cc[:], in1=tlo[:],
                                    op=OP.is_gt)
            chi = pp.tile([128, 1], I32)
            nc.vector.tensor_tensor(out=chi[:], in0=cc[:], in1=thi[:],
                                    op=OP.is_lt)
            ce_p = pp.tile([128, 1], F32)
            nc.vector.memset(ce_p[:], COS_EPS)
            ce_n = pp.tile([128, 1], F32)
            nc.vector.memset(ce_n[:], -COS_EPS)
            cm2 = pp.tile([128, 1], F32)
            nc.vector.select(cm2[:], clo[:], ce_p[:], cm[:])
            cm3 = pp.tile([128, 1], F32)
            nc.vector.select(cm3[:], chi[:], ce_n[:], cm2[:])
            w_in = pp.tile([128, 1], F32)
            nc.vector.tensor_scalar_mul(w_in[:], cm3[:], S)
            nc.vector.tensor_tensor(out=w_in[:], in0=w_in[:], in1=gadd64[:],
                                    op=OP.subtract)
            w_bf = pp.tile([128, 1], BF16)
            nc.vector.tensor_copy(w_bf[:], w_in[:])
            scat_sem = nc.alloc_semaphore("scat_sem")
            with tc.tile_critical(no_gpsimd_drain=True):
                nc.gpsimd.indirect_dma_start(
                    out=out_flat[:],
                    out_offset=IndirectOffsetOnAxis(ap=flat_m[:], axis=0),
                    in_=w_bf[:], in_offset=None,
                    bounds_check=CPAD * N - 1,
                    oob_is_err=False).then_inc(scat_sem, 16)

    nc.compile()
    return nc


def _get_nc():
    if "nc" not in _CACHE:
        _CACHE["nc"] = _build()
    return _CACHE["nc"]


def kernel(embeddings, kernel, norms, label):
    global LAST_RESULTS
    import ml_dtypes

    emb = np.asarray(embeddings, dtype=np.float32)
    kern_f = np.asarray(kernel, dtype=np.float32)
    norms_f = np.asarray(norms, dtype=np.float32)
    lab = np.asarray(label).astype(np.int64)

    nc = _get_nc()

    embT = np.ascontiguousarray(emb.T).astype(ml_dtypes.bfloat16)
    kern_bf = kern_f.astype(ml_dtypes.bfloat16)
    in_maps = []
    for i in range(NCORES):
        sh = np.ones((EMB, CPAD), dtype=ml_dtypes.bfloat16)
        sh[:, :CSH] = kern_bf[:, i * CSH:(i + 1) * CSH]
        c0 = i * CSH
        rows = np.where((lab >= c0) & (lab < c0 + CSH))[0]
        assert len(rows) <= 128, "fix list overflows one partition tile"
        ff = np.full((128, 1), 2 ** 30, dtype=np.int32)
        fn = np.full((128, 1), 10.0, dtype=np.float32)
        ff[:len(rows), 0] = ((lab[rows] - c0) * N + rows).astype(np.int32)
        fn[:len(rows), 0] = norms_f[rows]
        in_maps.append({
            "kern": sh,
            "embT": embT,
            "normv": norms_f,
            "fix_flat": ff,
            "fix_norm": fn,
        })

    trace = bool(os.environ.get("KBENCH_TRACE"))
    try:
        res = run_bass_kernel_spmd(nc, in_maps, core_ids=list(range(NCORES)),
                                   trace=trace)
    except ModuleNotFoundError:
        res = run_bass_kernel_spmd(nc, in_maps, core_ids=list(range(NCORES)))
    LAST_RESULTS = res

    scaled = np.empty((N, C), dtype=np.float32)
    for i in range(NCORES):
        scaled[:, i * CSH:(i + 1) * CSH] = \
            res.results[i]["out"][:CSH, :].astype(np.float32).T
    return scaled, emb
